# revision 27
# baseline (speedup 1.0000x reference)
"""Multi-head attention (B=2, QL=KL=2048, DIM=1024, H=16) on 8 TRN2 NeuronCores.

Sharding: core c handles batch c//4 and heads (c%4)*4 .. (c%4)*4+4 (column-
parallel q/k/v projections, row-parallel out projection). Each core emits a
partial output [QL, DIM]; the host sums the 4 partials per batch and adds the
output bias (the row-parallel all-reduce, done at unshard time).

Per-core kernel layout (all matmul operands bf16, fp32 PSUM accumulation):
  - activations are loaded feature-major (x^T tiles) via XBAR DMA-transpose
    (host pre-casts q/k/v to bf16; fp32 has no DMA-transpose path)
  - weights arrive host-pre-transposed (WqT etc), so no on-chip transposes
  - scores are computed transposed per head: S^T[j,i] = khT.T @ qhT (K=64)
  - exp(SCALE * S) is fused into the PSUM evacuation on the scalar engine
  - PV uses a ones-augmented V (lhsT [j, 65]) so row 64 of the PSUM output
    accumulates the softmax denominators for free
  - 1/sums via the 2-ULP custom-DVE reciprocal; the scale is applied to the
    fp32 attention output while casting it to bf16 for the out-projection
"""

import numpy as np
import ml_dtypes

import concourse.bass as bass
import concourse.mybir as mybir
import concourse.tile as tile
from concourse import bacc
from concourse.bass_utils import run_bass_kernel_spmd

BF16 = mybir.dt.bfloat16
F32 = mybir.dt.float32

B = 2
DIM = 1024
NUM_HEADS = 16
HD = DIM // NUM_HEADS  # 64
SCALE = HD ** -0.5
NCORES = 8
NH = 4          # heads per core
CDIM = NH * HD  # 256, per-core slice of the head dim
P = 128
IT = 512        # i (query) tile
ECH = DIM // P  # 8 contraction chunks for the projections


def build_bass(QL=2048, KL=2048, num_devices=NCORES):
    assert QL % IT == 0 and KL % 256 == 0
    NIT = QL // IT
    NJC = KL // P  # j (key) chunks

    nc = bacc.Bacc("TRN2", target_bir_lowering=False, debug=False,
                   num_devices=num_devices)
    qb = nc.dram_tensor("qb", [QL, DIM], BF16, kind="ExternalInput").ap()
    kb = nc.dram_tensor("kb", [KL, DIM], BF16, kind="ExternalInput").ap()
    vb = nc.dram_tensor("vb", [KL, DIM], BF16, kind="ExternalInput").ap()
    wqT = nc.dram_tensor("wqT", [DIM, CDIM], BF16, kind="ExternalInput").ap()
    wkT = nc.dram_tensor("wkT", [DIM, CDIM], BF16, kind="ExternalInput").ap()
    wvT = nc.dram_tensor("wvT", [DIM, CDIM], BF16, kind="ExternalInput").ap()
    woT = nc.dram_tensor("woT", [CDIM, DIM], BF16, kind="ExternalInput").ap()
    outp = nc.dram_tensor("outp", [QL, DIM], F32, kind="ExternalOutput").ap()

    with tile.TileContext(nc) as tc:
        with (
            tc.tile_pool(name="wpool", bufs=1) as wpool,
            tc.tile_pool(name="xpool", bufs=8) as xpool,
            tc.tile_pool(name="ppool", bufs=4) as ppool,
            tc.tile_pool(name="stpool", bufs=3) as stpool,
            tc.tile_pool(name="rpool", bufs=3) as rpool,
            tc.tile_pool(name="opool", bufs=4) as opool,
            tc.tile_pool(name="dpool", bufs=8, space="DRAM") as dpool,
            tc.tile_pool(name="psum", bufs=4, space="PSUM") as psum,
        ):
            # ---- persistent SBUF tensors ----
            wq_sb = wpool.tile([P, ECH, CDIM], BF16, tag="wq")
            wk_sb = wpool.tile([P, ECH, CDIM], BF16, tag="wk")
            wv_sb = wpool.tile([P, ECH, CDIM], BF16, tag="wv")
            wo_sb = wpool.tile([P, CDIM // P, DIM], BF16, tag="wo")
            nc.sync.dma_start(wq_sb[:], wqT.rearrange("(o p) d -> p o d", p=P))

            qhT = wpool.tile([P, CDIM // P, QL], BF16, tag="qhT")
            khT = wpool.tile([P, CDIM // P, KL], BF16, tag="khT")
            vh = wpool.tile([P, NJC, NH, HD + 1], BF16, tag="vh")
            nc.gpsimd.memset(vh[:, :, :, HD], 1.0)  # ones column -> row sums

            # ---- phase 1: q/k projections (out: [d'(256) part-major, token]) ----
            def proj_qk(x_dram, w_sb, dst, L):
                npairs = 2 * (L // IT)
                accs = [psum.tile([P, 2 * IT], F32, tag="ps", name=f"acc{i}")
                         for i in range((npairs + 1) // 2)]
                for e in range(ECH):
                    xT = xpool.tile([P, L], BF16, tag="xT")
                    nc.sync.dma_start_transpose(xT[:], x_dram[:, e * P:(e + 1) * P])
                    for d in range(2):
                        for it in range(L // IT):
                            pair = d * (L // IT) + it
                            tgt = accs[pair // 2][:, (pair % 2) * IT:(pair % 2 + 1) * IT]
                            nc.tensor.matmul(tgt, lhsT=w_sb[:, e, d * P:(d + 1) * P],
                                             rhs=xT[:, it * IT:(it + 1) * IT],
                                             start=(e == 0), stop=(e == ECH - 1))
                for d in range(2):
                    for it in range(L // IT):
                        pair = d * (L // IT) + it
                        src = accs[pair // 2][:, (pair % 2) * IT:(pair % 2 + 1) * IT]
                        dst_sl = dst[:, d, it * IT:(it + 1) * IT]
                        if it % 2 == 0:
                            nc.scalar.copy(dst_sl, src)
                        else:
                            nc.vector.tensor_copy(dst_sl, src)

            proj_qk(qb, wq_sb, qhT, QL)
            nc.sync.dma_start(wk_sb[:], wkT.rearrange("(o p) d -> p o d", p=P))
            proj_qk(kb, wk_sb, khT, KL)
            nc.sync.dma_start(wv_sb[:], wvT.rearrange("(o p) d -> p o d", p=P))

            # ---- v projection (out: [j part-major, head dim]) ----
            HALF = KL // 2
            for jg in range(2):
                njc_h = NJC // 2  # j-chunks in this half
                vaccs = [psum.tile([P, 2 * IT], F32, tag="ps", name=f"vacc{i}")
                         for i in range((njc_h + 1) // 2)]
                for e in range(ECH):
                    vT = xpool.tile([P, HALF], BF16, tag="xT")
                    nc.sync.dma_start_transpose(
                        vT[:], vb[jg * HALF:(jg + 1) * HALF, e * P:(e + 1) * P])
                    for jc in range(njc_h):
                        tgt = vaccs[jc // 2][:, (jc % 2) * IT:(jc % 2) * IT + CDIM]
                        nc.tensor.matmul(tgt, lhsT=vT[:, jc * P:(jc + 1) * P],
                                         rhs=wv_sb[:, e, :],
                                         start=(e == 0), stop=(e == ECH - 1))
                for jc in range(njc_h):
                    j = jg * njc_h + jc
                    src = vaccs[jc // 2][:, (jc % 2) * IT:(jc % 2) * IT + CDIM]
                    nc.vector.tensor_copy(
                        vh[:, j, :, 0:HD],
                        src.rearrange("p (h c) -> p h c", h=NH))

            nc.sync.dma_start(wo_sb[:], woT.rearrange("(o p) d -> p o d", p=P))

            # ---- phase 2: attention + fused softmax-rescale + out-proj ----
            # i-group outer (2 i-tiles = 1024 queries), heads inner; per
            # (h, ig) the inner j-loop is software-pipelined with LAG=2 so
            # the scalar-engine exp of chunk j overlaps the PV matmuls of
            # chunk j-2. The PV staging row 64 (softmax denominators) feeds
            # the custom-DVE reciprocal directly (partition 64 is a legal
            # start partition); the rescale happens before the partition-
            # redistributing DMA into the out-proj operand layout.
            LAG = 0
            attnTb = [wpool.tile([P, CDIM // P, 2 * IT], BF16, tag=f"attnTb{ig}",
                                 name=f"attnTb{ig}")
                      for ig in range(NIT // 2)]

            def emit_outproj(ig, icws):
                for icw in icws:
                    ic = ig * (2 * IT // P) + icw
                    po = psum.tile([P, 2 * IT], F32, tag="ps", name="po")
                    for ec in range(CDIM // P):
                        for dt in range(2):
                            nc.tensor.matmul(
                                po[:, dt * IT:(dt + 1) * IT],
                                lhsT=attnTb[ig][:, ec, icw * P:(icw + 1) * P],
                                rhs=wo_sb[:, ec, dt * IT:(dt + 1) * IT],
                                start=(ec == 0), stop=(ec == CDIM // P - 1))
                    for dt in range(2):
                        ob = opool.tile([P, IT], F32, tag="ob")
                        if (icw + dt) % 2 == 0:
                            nc.scalar.copy(ob[:], po[:, dt * IT:(dt + 1) * IT])
                        else:
                            nc.vector.tensor_copy(ob[:], po[:, dt * IT:(dt + 1) * IT])
                        nc.sync.dma_start(
                            outp[ic * P:(ic + 1) * P, dt * IT:(dt + 1) * IT], ob[:])
            for ig in range(NIT // 2):
                for hp in range(2):
                    pvs = [psum.tile([P, 2 * IT], F32, tag="ps", name=f"pv{hh}")
                           for hh in range(2)]
                    Pts = {}
                    for j in range(NJC + 1):
                        if j < NJC:
                            pss = [psum.tile([P, 2 * IT], F32, tag="ps",
                                             name=f"s{hh}") for hh in range(2)]
                            for itp in range(2):
                                it = ig * 2 + itp
                                for hh in range(2):
                                    sl = slice(hh * HD, (hh + 1) * HD)
                                    nc.tensor.matmul(
                                        pss[hh][:, itp * IT:(itp + 1) * IT],
                                        lhsT=khT[sl, hp, j * P:(j + 1) * P],
                                        rhs=qhT[sl, hp, it * IT:(it + 1) * IT],
                                        start=True, stop=True)
                            for hh in range(2):
                                Pt = ppool.tile([P, 2, IT], BF16, tag="Pt")
                                nc.scalar.activation(
                                    Pt[:],
                                    pss[hh][:].rearrange("p (a b) -> p a b", a=2),
                                    mybir.ActivationFunctionType.Exp, scale=SCALE)
                                Pts[(hh, j)] = Pt
                        if j >= 1:
                            jj = j - 1
                            for hh in range(2):
                                Pt = Pts.pop((hh, jj))
                                h = hp * 2 + hh
                                for itp in range(2):
                                    nc.tensor.matmul(
                                        pvs[hh][0:HD + 1, itp * IT:itp * IT + IT],
                                        lhsT=vh[:, jj, h, :], rhs=Pt[:, itp, :],
                                        start=(jj == 0), stop=(jj == NJC - 1))
                    # evacuate + rescale both heads of the pair
                    for hh in range(2):
                        st = stpool.tile([P, 2 * IT], F32, tag="pvstage")
                        nc.vector.tensor_copy(st[0:HD + 1, :], pvs[hh][0:HD + 1, :])
                        s0 = rpool.tile([1, 2 * IT], F32, tag="s0")
                        nc.sync.dma_start(s0[:], st[HD:HD + 1, :])
                        rrec = rpool.tile([1, 2 * IT], F32, tag="rrec")
                        rscr = rpool.tile([1, 2 * IT], F32, tag="rscr")
                        nc.vector.reciprocal_approx_accurate(
                            out=rrec[:], in_=s0[:], scratch=rscr[:])
                        dtmp = dpool.tile([1, 2 * IT], F32, tag="dtmp")
                        nc.sync.dma_start(dtmp[:], rrec[:])
                        rbc = rpool.tile([HD, 2 * IT], F32, tag="rbc")
                        nc.sync.dma_start(rbc[:],
                                          dtmp[0:1, :].broadcast_to((HD, 2 * IT)))
                        stb = stpool.tile([HD, 2 * IT], BF16, tag="stb")
                        nc.vector.tensor_mul(stb[:], st[0:HD, :], rbc[:])
                        nc.sync.dma_start(
                            attnTb[ig][hh * HD:(hh + 1) * HD, hp, :], stb[:])

                    if ig > 0:
                        emit_outproj(ig - 1, range(4 * hp, 4 * hp + 4))
                if ig == NIT // 2 - 1:
                    emit_outproj(ig, range(2 * IT // P))

    nc.compile()
    return nc


_NC_CACHE = {}


def _get_nc(QL, KL):
    key = (QL, KL)
    if key not in _NC_CACHE:
        _NC_CACHE[key] = build_bass(QL, KL)
    return _NC_CACHE[key]


def make_in_maps(q, k, v, Wq, Wk, Wv, Wo):
    """Per-core input maps (bf16, weights pre-transposed)."""
    bf = ml_dtypes.bfloat16
    q, k, v = (np.asarray(x, np.float32) for x in (q, k, v))
    WqT = np.asarray(Wq, np.float32).T.astype(bf)
    WkT = np.asarray(Wk, np.float32).T.astype(bf)
    WvT = np.asarray(Wv, np.float32).T.astype(bf)
    WoT = np.asarray(Wo, np.float32).T.astype(bf)
    qb = [np.ascontiguousarray(q[b].astype(bf)) for b in range(B)]
    kb = [np.ascontiguousarray(k[b].astype(bf)) for b in range(B)]
    vb = [np.ascontiguousarray(v[b].astype(bf)) for b in range(B)]
    in_maps = []
    for c in range(NCORES):
        b, hs = c // 4, c % 4
        sl = slice(hs * CDIM, (hs + 1) * CDIM)
        in_maps.append({
            "qb": qb[b], "kb": kb[b], "vb": vb[b],
            "wqT": np.ascontiguousarray(WqT[:, sl]),
            "wkT": np.ascontiguousarray(WkT[:, sl]),
            "wvT": np.ascontiguousarray(WvT[:, sl]),
            "woT": np.ascontiguousarray(WoT[sl, :]),
        })
    return in_maps


def kernel(q, k, v, Wq, Wk, Wv, Wo, bo, _trace=False):
    q = np.asarray(q, np.float32)
    QL, KL = q.shape[1], np.asarray(k).shape[1]
    nc = _get_nc(QL, KL)
    in_maps = make_in_maps(q, k, v, Wq, Wk, Wv, Wo)
    res = run_bass_kernel_spmd(nc, in_maps, core_ids=list(range(NCORES)),
                               trace=_trace)
    bo = np.asarray(bo, np.float32)
    out = np.empty((B, QL, DIM), np.float32)
    for b in range(B):
        acc = res.results[4 * b]["outp"].copy()
        for c in range(4 * b + 1, 4 * b + 4):
            acc += res.results[c]["outp"]
        out[b] = acc + bo
    if _trace:
        kernel._last_results = res
    return out


# revision 28
# speedup vs baseline: 1.2981x; 1.2981x over previous
"""Multi-head attention (B=2, QL=KL=2048, DIM=1024, H=16) on 8 TRN2 NeuronCores.

Sharding: core c handles batch c//4 and heads (c%4)*4 .. (c%4)*4+4 (column-
parallel q/k/v projections, row-parallel out projection). Each core emits a
partial output [QL, DIM]; the host sums the 4 partials per batch and adds the
output bias (the row-parallel all-reduce, done at unshard time).

Per-core kernel layout (all matmul operands bf16, fp32 PSUM accumulation):
  - activations are loaded feature-major (x^T tiles) via XBAR DMA-transpose
    (host pre-casts q/k/v to bf16; fp32 has no DMA-transpose path)
  - weights arrive host-pre-transposed (WqT etc), so no on-chip transposes
  - scores are computed transposed per head: S^T[j,i] = khT.T @ qhT (K=64)
  - exp(SCALE * S) is fused into the PSUM evacuation on the scalar engine
  - PV uses a ones-augmented V (lhsT [j, 65]) so row 64 of the PSUM output
    accumulates the softmax denominators for free
  - 1/sums via the 2-ULP custom-DVE reciprocal; the scale is applied to the
    fp32 attention output while casting it to bf16 for the out-projection
"""

import numpy as np
import ml_dtypes

import concourse.bass as bass
import concourse.mybir as mybir
import concourse.tile as tile
from concourse import bacc
from concourse.bass_utils import run_bass_kernel_spmd

BF16 = mybir.dt.bfloat16
F32 = mybir.dt.float32

B = 2
DIM = 1024
NUM_HEADS = 16
HD = DIM // NUM_HEADS  # 64
SCALE = HD ** -0.5
NCORES = 8
NH = 4          # heads per core
CDIM = NH * HD  # 256, per-core slice of the head dim
P = 128
IT = 512        # i (query) tile
ECH = DIM // P  # 8 contraction chunks for the projections


def build_bass(QL=2048, KL=2048, num_devices=NCORES):
    assert QL % IT == 0 and KL % 256 == 0
    NIT = QL // IT
    NJC = KL // P  # j (key) chunks

    nc = bacc.Bacc("TRN2", target_bir_lowering=False, debug=False,
                   num_devices=num_devices)
    qb = nc.dram_tensor("qb", [QL, DIM], BF16, kind="ExternalInput").ap()
    kb = nc.dram_tensor("kb", [KL, DIM], BF16, kind="ExternalInput").ap()
    vb = nc.dram_tensor("vb", [KL, DIM], BF16, kind="ExternalInput").ap()
    wqT = nc.dram_tensor("wqT", [DIM, CDIM], BF16, kind="ExternalInput").ap()
    wkT = nc.dram_tensor("wkT", [DIM, CDIM], BF16, kind="ExternalInput").ap()
    wvT = nc.dram_tensor("wvT", [DIM, CDIM], BF16, kind="ExternalInput").ap()
    woT = nc.dram_tensor("woT", [CDIM, DIM], BF16, kind="ExternalInput").ap()
    outp = nc.dram_tensor("outp", [QL, DIM], F32, kind="ExternalOutput").ap()

    with tile.TileContext(nc) as tc:
        with (
            tc.tile_pool(name="wpool", bufs=1) as wpool,
            tc.tile_pool(name="xpool", bufs=8) as xpool,
            tc.tile_pool(name="ppool", bufs=4) as ppool,
            tc.tile_pool(name="stpool", bufs=3) as stpool,
            tc.tile_pool(name="rpool", bufs=3) as rpool,
            tc.tile_pool(name="opool", bufs=4) as opool,
            tc.tile_pool(name="dpool", bufs=8, space="DRAM") as dpool,
            tc.tile_pool(name="psum", bufs=4, space="PSUM") as psum,
        ):
            # ---- persistent SBUF tensors ----
            wq_sb = wpool.tile([P, ECH, CDIM], BF16, tag="wq")
            wk_sb = wpool.tile([P, ECH, CDIM], BF16, tag="wk")
            wv_sb = wpool.tile([P, ECH, CDIM], BF16, tag="wv")
            wo_sb = wpool.tile([P, CDIM // P, DIM], BF16, tag="wo")
            nc.sync.dma_start(wq_sb[:], wqT.rearrange("(o p) d -> p o d", p=P))

            qhT = wpool.tile([P, CDIM // P, QL], BF16, tag="qhT")
            khT = wpool.tile([P, CDIM // P, KL], BF16, tag="khT")
            vh = wpool.tile([P, NJC, NH, HD + 1], BF16, tag="vh")
            nc.gpsimd.memset(vh[:, :, :, HD], 1.0)  # ones column -> row sums

            # ---- phase 1: q/k projections (out: [d'(256) part-major, token]) ----
            def proj_qk(x_dram, w_sb, dst, L):
                npairs = 2 * (L // IT)
                accs = [psum.tile([P, 2 * IT], F32, tag="ps", name=f"acc{i}")
                         for i in range((npairs + 1) // 2)]
                for e in range(ECH):
                    xT = xpool.tile([P, L], BF16, tag="xT")
                    nc.sync.dma_start_transpose(xT[:], x_dram[:, e * P:(e + 1) * P])
                    for d in range(2):
                        for it in range(L // IT):
                            pair = d * (L // IT) + it
                            tgt = accs[pair // 2][:, (pair % 2) * IT:(pair % 2 + 1) * IT]
                            nc.tensor.matmul(tgt, lhsT=w_sb[:, e, d * P:(d + 1) * P],
                                             rhs=xT[:, it * IT:(it + 1) * IT],
                                             start=(e == 0), stop=(e == ECH - 1))
                for d in range(2):
                    for it in range(L // IT):
                        pair = d * (L // IT) + it
                        src = accs[pair // 2][:, (pair % 2) * IT:(pair % 2 + 1) * IT]
                        dst_sl = dst[:, d, it * IT:(it + 1) * IT]
                        if it % 2 == 0:
                            nc.scalar.copy(dst_sl, src)
                        else:
                            nc.vector.tensor_copy(dst_sl, src)

            proj_qk(qb, wq_sb, qhT, QL)
            nc.sync.dma_start(wk_sb[:], wkT.rearrange("(o p) d -> p o d", p=P))
            proj_qk(kb, wk_sb, khT, KL)
            nc.sync.dma_start(wv_sb[:], wvT.rearrange("(o p) d -> p o d", p=P))

            # ---- v projection (out: [j part-major, head dim]) ----
            HALF = KL // 2
            for jg in range(2):
                njc_h = NJC // 2  # j-chunks in this half
                vaccs = [psum.tile([P, 2 * IT], F32, tag="ps", name=f"vacc{i}")
                         for i in range((njc_h + 1) // 2)]
                for e in range(ECH):
                    vT = xpool.tile([P, HALF], BF16, tag="xT")
                    nc.sync.dma_start_transpose(
                        vT[:], vb[jg * HALF:(jg + 1) * HALF, e * P:(e + 1) * P])
                    for jc in range(njc_h):
                        tgt = vaccs[jc // 2][:, (jc % 2) * IT:(jc % 2) * IT + CDIM]
                        nc.tensor.matmul(tgt, lhsT=vT[:, jc * P:(jc + 1) * P],
                                         rhs=wv_sb[:, e, :],
                                         start=(e == 0), stop=(e == ECH - 1))
                for jc in range(njc_h):
                    j = jg * njc_h + jc
                    src = vaccs[jc // 2][:, (jc % 2) * IT:(jc % 2) * IT + CDIM]
                    nc.vector.tensor_copy(
                        vh[:, j, :, 0:HD],
                        src.rearrange("p (h c) -> p h c", h=NH))

            nc.sync.dma_start(wo_sb[:], woT.rearrange("(o p) d -> p o d", p=P))

            # ---- phase 2: attention + fused softmax-rescale + out-proj ----
            # i-group outer (2 i-tiles = 1024 queries), heads inner; per
            # (h, ig) the inner j-loop is software-pipelined with LAG=2 so
            # the scalar-engine exp of chunk j overlaps the PV matmuls of
            # chunk j-2. The PV staging row 64 (softmax denominators) feeds
            # the custom-DVE reciprocal directly (partition 64 is a legal
            # start partition); the rescale happens before the partition-
            # redistributing DMA into the out-proj operand layout.
            LAG = 0
            attnTb = [wpool.tile([P, CDIM // P, 2 * IT], BF16, tag=f"attnTb{ig}",
                                 name=f"attnTb{ig}")
                      for ig in range(NIT // 2)]

            def emit_outproj(ig, icws):
                for icw in icws:
                    ic = ig * (2 * IT // P) + icw
                    po = psum.tile([P, 2 * IT], F32, tag="ps", name="po")
                    for ec in range(CDIM // P):
                        for dt in range(2):
                            nc.tensor.matmul(
                                po[:, dt * IT:(dt + 1) * IT],
                                lhsT=attnTb[ig][:, ec, icw * P:(icw + 1) * P],
                                rhs=wo_sb[:, ec, dt * IT:(dt + 1) * IT],
                                start=(ec == 0), stop=(ec == CDIM // P - 1))
                    for dt in range(2):
                        ob = opool.tile([P, IT], F32, tag="ob")
                        if (icw + dt) % 2 == 0:
                            nc.scalar.copy(ob[:], po[:, dt * IT:(dt + 1) * IT])
                        else:
                            nc.vector.tensor_copy(ob[:], po[:, dt * IT:(dt + 1) * IT])
                        nc.sync.dma_start(
                            outp[ic * P:(ic + 1) * P, dt * IT:(dt + 1) * IT], ob[:])
            for ig in range(NIT // 2):
                for h in range(NH):
                    hp, hh = h // 2, h % 2
                    q_h = qhT[hh * HD:(hh + 1) * HD, hp, :]
                    k_h = khT[hh * HD:(hh + 1) * HD, hp, :]
                    pv = psum.tile([P, 2 * IT], F32, tag="ps", name="pv")
                    Pts = {}
                    NB = NJC // 2  # batches of 2 j-chunks, PV trails one batch
                    for b in range(NB + 1):
                        if b < NB:
                            pss = []
                            for jo in range(2):
                                j = 2 * b + jo
                                ps = psum.tile([P, 2 * IT], F32, tag="ps",
                                               name="s")
                                pss.append(ps)
                                for itp in range(2):
                                    it = ig * 2 + itp
                                    nc.tensor.matmul(
                                        ps[:, itp * IT:(itp + 1) * IT],
                                        lhsT=k_h[:, j * P:(j + 1) * P],
                                        rhs=q_h[:, it * IT:(it + 1) * IT],
                                        start=True, stop=True)
                            for jo in range(2):
                                j = 2 * b + jo
                                Pt = ppool.tile([P, 2, IT], BF16, tag="Pt")
                                nc.scalar.activation(
                                    Pt[:],
                                    pss[jo][:].rearrange("p (a b) -> p a b", a=2),
                                    mybir.ActivationFunctionType.Exp, scale=SCALE)
                                Pts[j] = Pt
                        if b >= 1:
                            for jo in range(2):
                                jj = 2 * (b - 1) + jo
                                Pt = Pts.pop(jj)
                                for itp in range(2):
                                    nc.tensor.matmul(
                                        pv[0:HD + 1, itp * IT:itp * IT + IT],
                                        lhsT=vh[:, jj, h, :], rhs=Pt[:, itp, :],
                                        start=(jj == 0), stop=(jj == NJC - 1))
                    # evacuate + rescale: st rows 0..63 = PV, row 64 = sums
                    st = stpool.tile([P, 2 * IT], F32, tag="pvstage")
                    nc.vector.tensor_copy(st[0:HD + 1, :], pv[0:HD + 1, :])
                    s0 = rpool.tile([1, 2 * IT], F32, tag="s0")
                    nc.sync.dma_start(s0[:], st[HD:HD + 1, :])
                    rrec = rpool.tile([1, 2 * IT], F32, tag="rrec")
                    rscr = rpool.tile([1, 2 * IT], F32, tag="rscr")
                    nc.vector.reciprocal_approx_accurate(
                        out=rrec[:], in_=s0[:], scratch=rscr[:])
                    dtmp = dpool.tile([1, 2 * IT], F32, tag="dtmp")
                    nc.sync.dma_start(dtmp[:], rrec[:])
                    rbc = rpool.tile([HD, 2 * IT], F32, tag="rbc")
                    nc.sync.dma_start(rbc[:], dtmp[0:1, :].broadcast_to((HD, 2 * IT)))
                    stb = stpool.tile([HD, 2 * IT], BF16, tag="stb")
                    nc.vector.tensor_mul(stb[:], st[0:HD, :], rbc[:])
                    nc.sync.dma_start(
                        attnTb[ig][hh * HD:(hh + 1) * HD, hp, :], stb[:])

                    if ig > 0:
                        emit_outproj(ig - 1, range(2 * h, 2 * h + 2))
                if ig == NIT // 2 - 1:
                    emit_outproj(ig, range(2 * IT // P))

    nc.compile()
    return nc


_NC_CACHE = {}


def _get_nc(QL, KL):
    key = (QL, KL)
    if key not in _NC_CACHE:
        _NC_CACHE[key] = build_bass(QL, KL)
    return _NC_CACHE[key]


def make_in_maps(q, k, v, Wq, Wk, Wv, Wo):
    """Per-core input maps (bf16, weights pre-transposed)."""
    bf = ml_dtypes.bfloat16
    q, k, v = (np.asarray(x, np.float32) for x in (q, k, v))
    WqT = np.asarray(Wq, np.float32).T.astype(bf)
    WkT = np.asarray(Wk, np.float32).T.astype(bf)
    WvT = np.asarray(Wv, np.float32).T.astype(bf)
    WoT = np.asarray(Wo, np.float32).T.astype(bf)
    qb = [np.ascontiguousarray(q[b].astype(bf)) for b in range(B)]
    kb = [np.ascontiguousarray(k[b].astype(bf)) for b in range(B)]
    vb = [np.ascontiguousarray(v[b].astype(bf)) for b in range(B)]
    in_maps = []
    for c in range(NCORES):
        b, hs = c // 4, c % 4
        sl = slice(hs * CDIM, (hs + 1) * CDIM)
        in_maps.append({
            "qb": qb[b], "kb": kb[b], "vb": vb[b],
            "wqT": np.ascontiguousarray(WqT[:, sl]),
            "wkT": np.ascontiguousarray(WkT[:, sl]),
            "wvT": np.ascontiguousarray(WvT[:, sl]),
            "woT": np.ascontiguousarray(WoT[sl, :]),
        })
    return in_maps


def kernel(q, k, v, Wq, Wk, Wv, Wo, bo, _trace=False):
    q = np.asarray(q, np.float32)
    QL, KL = q.shape[1], np.asarray(k).shape[1]
    nc = _get_nc(QL, KL)
    in_maps = make_in_maps(q, k, v, Wq, Wk, Wv, Wo)
    res = run_bass_kernel_spmd(nc, in_maps, core_ids=list(range(NCORES)),
                               trace=_trace)
    bo = np.asarray(bo, np.float32)
    out = np.empty((B, QL, DIM), np.float32)
    for b in range(B):
        acc = res.results[4 * b]["outp"].copy()
        for c in range(4 * b + 1, 4 * b + 4):
            acc += res.results[c]["outp"]
        out[b] = acc + bo
    if _trace:
        kernel._last_results = res
    return out


# revision 29
# speedup vs baseline: 1.3108x; 1.0098x over previous
"""Multi-head attention (B=2, QL=KL=2048, DIM=1024, H=16) on 8 TRN2 NeuronCores.

Sharding: core c handles batch c//4 and heads (c%4)*4 .. (c%4)*4+4 (column-
parallel q/k/v projections, row-parallel out projection). Each core emits a
partial output [QL, DIM]; the host sums the 4 partials per batch and adds the
output bias (the row-parallel all-reduce, done at unshard time).

Per-core kernel layout (all matmul operands bf16, fp32 PSUM accumulation):
  - activations are loaded feature-major (x^T tiles) via XBAR DMA-transpose
    (host pre-casts q/k/v to bf16; fp32 has no DMA-transpose path)
  - weights arrive host-pre-transposed (WqT etc), so no on-chip transposes
  - scores are computed transposed per head: S^T[j,i] = khT.T @ qhT (K=64)
  - exp(SCALE * S) is fused into the PSUM evacuation on the scalar engine
  - PV uses a ones-augmented V (lhsT [j, 65]) so row 64 of the PSUM output
    accumulates the softmax denominators for free
  - 1/sums via the 2-ULP custom-DVE reciprocal; the scale is applied to the
    fp32 attention output while casting it to bf16 for the out-projection
"""

import numpy as np
import ml_dtypes

import concourse.bass as bass
import concourse.mybir as mybir
import concourse.tile as tile
from concourse import bacc
from concourse.bass_utils import run_bass_kernel_spmd

BF16 = mybir.dt.bfloat16
F32 = mybir.dt.float32

B = 2
DIM = 1024
NUM_HEADS = 16
HD = DIM // NUM_HEADS  # 64
SCALE = HD ** -0.5
NCORES = 8
NH = 4          # heads per core
CDIM = NH * HD  # 256, per-core slice of the head dim
P = 128
IT = 512        # i (query) tile
ECH = DIM // P  # 8 contraction chunks for the projections


def build_bass(QL=2048, KL=2048, num_devices=NCORES):
    assert QL % IT == 0 and KL % 256 == 0
    NIT = QL // IT
    NJC = KL // P  # j (key) chunks

    nc = bacc.Bacc("TRN2", target_bir_lowering=False, debug=False,
                   num_devices=num_devices)
    qb = nc.dram_tensor("qb", [QL, DIM], BF16, kind="ExternalInput").ap()
    kb = nc.dram_tensor("kb", [KL, DIM], BF16, kind="ExternalInput").ap()
    vb = nc.dram_tensor("vb", [KL, DIM], BF16, kind="ExternalInput").ap()
    wqT = nc.dram_tensor("wqT", [DIM, CDIM], BF16, kind="ExternalInput").ap()
    wkT = nc.dram_tensor("wkT", [DIM, CDIM], BF16, kind="ExternalInput").ap()
    wvT = nc.dram_tensor("wvT", [DIM, CDIM], BF16, kind="ExternalInput").ap()
    woT = nc.dram_tensor("woT", [CDIM, DIM], BF16, kind="ExternalInput").ap()
    outp = nc.dram_tensor("outp", [QL, DIM], F32, kind="ExternalOutput").ap()

    with tile.TileContext(nc) as tc:
        with (
            tc.tile_pool(name="wpool", bufs=1) as wpool,
            tc.tile_pool(name="xpool", bufs=8) as xpool,
            tc.tile_pool(name="ppool", bufs=4) as ppool,
            tc.tile_pool(name="stpool", bufs=3) as stpool,
            tc.tile_pool(name="rpool", bufs=3) as rpool,
            tc.tile_pool(name="opool", bufs=4) as opool,
            tc.tile_pool(name="dpool", bufs=8, space="DRAM") as dpool,
            tc.tile_pool(name="psum", bufs=4, space="PSUM") as psum,
        ):
            # ---- persistent SBUF tensors ----
            wq_sb = wpool.tile([P, ECH, CDIM], BF16, tag="wq")
            wk_sb = wpool.tile([P, ECH, CDIM], BF16, tag="wk")
            wv_sb = wpool.tile([P, ECH, CDIM], BF16, tag="wv")
            wo_sb = wpool.tile([P, CDIM // P, DIM], BF16, tag="wo")
            nc.sync.dma_start(wq_sb[:], wqT.rearrange("(o p) d -> p o d", p=P))

            qhT = wpool.tile([P, CDIM // P, QL], BF16, tag="qhT")
            khT = wpool.tile([P, CDIM // P, KL], BF16, tag="khT")
            vh = wpool.tile([P, NJC, NH, HD + 1], BF16, tag="vh")
            nc.gpsimd.memset(vh[:, :, :, HD], 1.0)  # ones column -> row sums

            # ---- phase 1: q/k projections (out: [d'(256) part-major, token]) ----
            def proj_qk(x_dram, w_sb, dst, L):
                npairs = 2 * (L // IT)
                accs = [psum.tile([P, 2 * IT], F32, tag="ps", name=f"acc{i}")
                         for i in range((npairs + 1) // 2)]
                for e in range(ECH):
                    xT = xpool.tile([P, L], BF16, tag="xT")
                    nc.sync.dma_start_transpose(xT[:], x_dram[:, e * P:(e + 1) * P])
                    for d in range(2):
                        for it in range(L // IT):
                            pair = d * (L // IT) + it
                            tgt = accs[pair // 2][:, (pair % 2) * IT:(pair % 2 + 1) * IT]
                            nc.tensor.matmul(tgt, lhsT=w_sb[:, e, d * P:(d + 1) * P],
                                             rhs=xT[:, it * IT:(it + 1) * IT],
                                             start=(e == 0), stop=(e == ECH - 1))
                for d in range(2):
                    for it in range(L // IT):
                        pair = d * (L // IT) + it
                        src = accs[pair // 2][:, (pair % 2) * IT:(pair % 2 + 1) * IT]
                        dst_sl = dst[:, d, it * IT:(it + 1) * IT]
                        if it % 2 == 0:
                            nc.scalar.copy(dst_sl, src)
                        else:
                            nc.vector.tensor_copy(dst_sl, src)

            proj_qk(qb, wq_sb, qhT, QL)
            nc.scalar.dma_start(wk_sb[:], wkT.rearrange("(o p) d -> p o d", p=P))
            proj_qk(kb, wk_sb, khT, KL)
            nc.scalar.dma_start(wv_sb[:], wvT.rearrange("(o p) d -> p o d", p=P))

            # ---- v projection (out: [j part-major, head dim]) ----
            HALF = KL // 2
            for jg in range(2):
                njc_h = NJC // 2  # j-chunks in this half
                vaccs = [psum.tile([P, 2 * IT], F32, tag="ps", name=f"vacc{i}")
                         for i in range((njc_h + 1) // 2)]
                for e in range(ECH):
                    vT = xpool.tile([P, HALF], BF16, tag="xT")
                    nc.sync.dma_start_transpose(
                        vT[:], vb[jg * HALF:(jg + 1) * HALF, e * P:(e + 1) * P])
                    for jc in range(njc_h):
                        tgt = vaccs[jc // 2][:, (jc % 2) * IT:(jc % 2) * IT + CDIM]
                        nc.tensor.matmul(tgt, lhsT=vT[:, jc * P:(jc + 1) * P],
                                         rhs=wv_sb[:, e, :],
                                         start=(e == 0), stop=(e == ECH - 1))
                for jc in range(njc_h):
                    j = jg * njc_h + jc
                    src = vaccs[jc // 2][:, (jc % 2) * IT:(jc % 2) * IT + CDIM]
                    nc.vector.tensor_copy(
                        vh[:, j, :, 0:HD],
                        src.rearrange("p (h c) -> p h c", h=NH))

            nc.scalar.dma_start(wo_sb[:], woT.rearrange("(o p) d -> p o d", p=P))

            # ---- phase 2: attention + fused softmax-rescale + out-proj ----
            # i-group outer (2 i-tiles = 1024 queries), heads inner; per
            # (h, ig) the inner j-loop is software-pipelined with LAG=2 so
            # the scalar-engine exp of chunk j overlaps the PV matmuls of
            # chunk j-2. The PV staging row 64 (softmax denominators) feeds
            # the custom-DVE reciprocal directly (partition 64 is a legal
            # start partition); the rescale happens before the partition-
            # redistributing DMA into the out-proj operand layout.
            LAG = 0
            attnTb = [wpool.tile([P, CDIM // P, 2 * IT], BF16, tag=f"attnTb{ig}",
                                 name=f"attnTb{ig}")
                      for ig in range(NIT // 2)]

            def emit_outproj(ig, icws):
                for icw in icws:
                    ic = ig * (2 * IT // P) + icw
                    po = psum.tile([P, 2 * IT], F32, tag="ps", name="po")
                    for ec in range(CDIM // P):
                        for dt in range(2):
                            nc.tensor.matmul(
                                po[:, dt * IT:(dt + 1) * IT],
                                lhsT=attnTb[ig][:, ec, icw * P:(icw + 1) * P],
                                rhs=wo_sb[:, ec, dt * IT:(dt + 1) * IT],
                                start=(ec == 0), stop=(ec == CDIM // P - 1))
                    for dt in range(2):
                        ob = opool.tile([P, IT], F32, tag="ob")
                        if (icw + dt) % 2 == 0:
                            nc.scalar.copy(ob[:], po[:, dt * IT:(dt + 1) * IT])
                        else:
                            nc.vector.tensor_copy(ob[:], po[:, dt * IT:(dt + 1) * IT])
                        nc.scalar.dma_start(
                            outp[ic * P:(ic + 1) * P, dt * IT:(dt + 1) * IT], ob[:])
            for ig in range(NIT // 2):
                for h in range(NH):
                    hp, hh = h // 2, h % 2
                    q_h = qhT[hh * HD:(hh + 1) * HD, hp, :]
                    k_h = khT[hh * HD:(hh + 1) * HD, hp, :]
                    pv = psum.tile([P, 2 * IT], F32, tag="ps", name="pv")
                    Pts = {}
                    NB = NJC // 2  # batches of 2 j-chunks, PV trails one batch
                    for b in range(NB + 1):
                        if b < NB:
                            pss = []
                            for jo in range(2):
                                j = 2 * b + jo
                                ps = psum.tile([P, 2 * IT], F32, tag="ps",
                                               name="s")
                                pss.append(ps)
                                for itp in range(2):
                                    it = ig * 2 + itp
                                    nc.tensor.matmul(
                                        ps[:, itp * IT:(itp + 1) * IT],
                                        lhsT=k_h[:, j * P:(j + 1) * P],
                                        rhs=q_h[:, it * IT:(it + 1) * IT],
                                        start=True, stop=True)
                            for jo in range(2):
                                j = 2 * b + jo
                                Pt = ppool.tile([P, 2, IT], BF16, tag="Pt")
                                nc.scalar.activation(
                                    Pt[:],
                                    pss[jo][:].rearrange("p (a b) -> p a b", a=2),
                                    mybir.ActivationFunctionType.Exp, scale=SCALE)
                                Pts[j] = Pt
                        if b >= 1:
                            for jo in range(2):
                                jj = 2 * (b - 1) + jo
                                Pt = Pts.pop(jj)
                                for itp in range(2):
                                    nc.tensor.matmul(
                                        pv[0:HD + 1, itp * IT:itp * IT + IT],
                                        lhsT=vh[:, jj, h, :], rhs=Pt[:, itp, :],
                                        start=(jj == 0), stop=(jj == NJC - 1))
                    # evacuate + rescale: st rows 0..63 = PV, row 64 = sums
                    st = stpool.tile([P, 2 * IT], F32, tag="pvstage")
                    nc.vector.tensor_copy(st[0:HD + 1, :], pv[0:HD + 1, :])
                    s0 = rpool.tile([1, 2 * IT], F32, tag="s0")
                    nc.sync.dma_start(s0[:], st[HD:HD + 1, :])
                    rrec = rpool.tile([1, 2 * IT], F32, tag="rrec")
                    rscr = rpool.tile([1, 2 * IT], F32, tag="rscr")
                    nc.vector.reciprocal_approx_accurate(
                        out=rrec[:], in_=s0[:], scratch=rscr[:])
                    dtmp = dpool.tile([1, 2 * IT], F32, tag="dtmp")
                    nc.sync.dma_start(dtmp[:], rrec[:])
                    rbc = rpool.tile([HD, 2 * IT], F32, tag="rbc")
                    nc.sync.dma_start(rbc[:], dtmp[0:1, :].broadcast_to((HD, 2 * IT)))
                    stb = stpool.tile([HD, 2 * IT], BF16, tag="stb")
                    nc.vector.tensor_mul(stb[:], st[0:HD, :], rbc[:])
                    nc.sync.dma_start(
                        attnTb[ig][hh * HD:(hh + 1) * HD, hp, :], stb[:])

                    if ig > 0:
                        emit_outproj(ig - 1, range(2 * h, 2 * h + 2))
                if ig == NIT // 2 - 1:
                    emit_outproj(ig, range(2 * IT // P))

    nc.compile()
    return nc


_NC_CACHE = {}


def _get_nc(QL, KL):
    key = (QL, KL)
    if key not in _NC_CACHE:
        _NC_CACHE[key] = build_bass(QL, KL)
    return _NC_CACHE[key]


def make_in_maps(q, k, v, Wq, Wk, Wv, Wo):
    """Per-core input maps (bf16, weights pre-transposed)."""
    bf = ml_dtypes.bfloat16
    q, k, v = (np.asarray(x, np.float32) for x in (q, k, v))
    WqT = np.asarray(Wq, np.float32).T.astype(bf)
    WkT = np.asarray(Wk, np.float32).T.astype(bf)
    WvT = np.asarray(Wv, np.float32).T.astype(bf)
    WoT = np.asarray(Wo, np.float32).T.astype(bf)
    qb = [np.ascontiguousarray(q[b].astype(bf)) for b in range(B)]
    kb = [np.ascontiguousarray(k[b].astype(bf)) for b in range(B)]
    vb = [np.ascontiguousarray(v[b].astype(bf)) for b in range(B)]
    in_maps = []
    for c in range(NCORES):
        b, hs = c // 4, c % 4
        sl = slice(hs * CDIM, (hs + 1) * CDIM)
        in_maps.append({
            "qb": qb[b], "kb": kb[b], "vb": vb[b],
            "wqT": np.ascontiguousarray(WqT[:, sl]),
            "wkT": np.ascontiguousarray(WkT[:, sl]),
            "wvT": np.ascontiguousarray(WvT[:, sl]),
            "woT": np.ascontiguousarray(WoT[sl, :]),
        })
    return in_maps


def kernel(q, k, v, Wq, Wk, Wv, Wo, bo, _trace=False):
    q = np.asarray(q, np.float32)
    QL, KL = q.shape[1], np.asarray(k).shape[1]
    nc = _get_nc(QL, KL)
    in_maps = make_in_maps(q, k, v, Wq, Wk, Wv, Wo)
    res = run_bass_kernel_spmd(nc, in_maps, core_ids=list(range(NCORES)),
                               trace=_trace)
    bo = np.asarray(bo, np.float32)
    out = np.empty((B, QL, DIM), np.float32)
    for b in range(B):
        acc = res.results[4 * b]["outp"].copy()
        for c in range(4 * b + 1, 4 * b + 4):
            acc += res.results[c]["outp"]
        out[b] = acc + bo
    if _trace:
        kernel._last_results = res
    return out


# revision 30
# speedup vs baseline: 1.4174x; 1.0813x over previous
"""Multi-head attention (B=2, QL=KL=2048, DIM=1024, H=16) on 8 TRN2 NeuronCores.

Sharding: core c handles batch c//4 and heads (c%4)*4 .. (c%4)*4+4 (column-
parallel q/k/v projections, row-parallel out projection). Each core emits a
partial output [QL, DIM]; the host sums the 4 partials per batch and adds the
output bias (the row-parallel all-reduce, done at unshard time).

Per-core kernel layout (all matmul operands bf16, fp32 PSUM accumulation):
  - activations are loaded feature-major (x^T tiles) via XBAR DMA-transpose
    (host pre-casts q/k/v to bf16; fp32 has no DMA-transpose path)
  - weights arrive host-pre-transposed (WqT etc), so no on-chip transposes
  - scores are computed transposed per head: S^T[j,i] = khT.T @ qhT (K=64)
  - exp(SCALE * S) is fused into the PSUM evacuation on the scalar engine
  - PV uses a ones-augmented V (lhsT [j, 65]) so row 64 of the PSUM output
    accumulates the softmax denominators for free
  - 1/sums via the 2-ULP custom-DVE reciprocal; the scale is applied to the
    fp32 attention output while casting it to bf16 for the out-projection
"""

import numpy as np
import ml_dtypes

import concourse.bass as bass
import concourse.mybir as mybir
import concourse.tile as tile
from concourse import bacc
from concourse.bass_utils import run_bass_kernel_spmd

BF16 = mybir.dt.bfloat16
F32 = mybir.dt.float32

B = 2
DIM = 1024
NUM_HEADS = 16
HD = DIM // NUM_HEADS  # 64
SCALE = HD ** -0.5
NCORES = 8
NH = 4          # heads per core
CDIM = NH * HD  # 256, per-core slice of the head dim
P = 128
IT = 512        # i (query) tile
ECH = DIM // P  # 8 contraction chunks for the projections


def build_bass(QL=2048, KL=2048, num_devices=NCORES):
    assert QL % IT == 0 and KL % 256 == 0
    NIT = QL // IT
    NJC = KL // P  # j (key) chunks

    nc = bacc.Bacc("TRN2", target_bir_lowering=False, debug=False,
                   num_devices=num_devices)
    qb = nc.dram_tensor("qbT", [DIM, QL], BF16, kind="ExternalInput").ap()
    kb = nc.dram_tensor("kbT", [DIM, KL], BF16, kind="ExternalInput").ap()
    vb = nc.dram_tensor("vbT", [DIM, KL], BF16, kind="ExternalInput").ap()
    wqT = nc.dram_tensor("wqT", [DIM, CDIM], BF16, kind="ExternalInput").ap()
    wkT = nc.dram_tensor("wkT", [DIM, CDIM], BF16, kind="ExternalInput").ap()
    wvT = nc.dram_tensor("wvT", [DIM, CDIM], BF16, kind="ExternalInput").ap()
    woT = nc.dram_tensor("woT", [CDIM, DIM], BF16, kind="ExternalInput").ap()
    outp = nc.dram_tensor("outp", [QL, DIM], F32, kind="ExternalOutput").ap()

    with tile.TileContext(nc) as tc:
        with (
            tc.tile_pool(name="wpool", bufs=1) as wpool,
            tc.tile_pool(name="xpool", bufs=8) as xpool,
            tc.tile_pool(name="ppool", bufs=4) as ppool,
            tc.tile_pool(name="stpool", bufs=3) as stpool,
            tc.tile_pool(name="rpool", bufs=3) as rpool,
            tc.tile_pool(name="opool", bufs=4) as opool,
            tc.tile_pool(name="dpool", bufs=8, space="DRAM") as dpool,
            tc.tile_pool(name="psum", bufs=4, space="PSUM") as psum,
        ):
            # ---- persistent SBUF tensors ----
            wq_sb = wpool.tile([P, ECH, CDIM], BF16, tag="wq")
            wk_sb = wpool.tile([P, ECH, CDIM], BF16, tag="wk")
            wv_sb = wpool.tile([P, ECH, CDIM], BF16, tag="wv")
            wo_sb = wpool.tile([P, CDIM // P, DIM], BF16, tag="wo")
            nc.sync.dma_start(wq_sb[:], wqT.rearrange("(o p) d -> p o d", p=P))

            qhT = wpool.tile([P, CDIM // P, QL], BF16, tag="qhT")
            khT = wpool.tile([P, CDIM // P, KL], BF16, tag="khT")
            vh = wpool.tile([P, NJC, NH, HD + 1], BF16, tag="vh")
            nc.gpsimd.memset(vh[:, :, :, HD], 1.0)  # ones column -> row sums

            # ---- phase 1: q/k projections (out: [d'(256) part-major, token]) ----
            def proj_qk(x_dram, w_sb, dst, L):
                npairs = 2 * (L // IT)
                accs = [psum.tile([P, 2 * IT], F32, tag="ps", name=f"acc{i}")
                         for i in range((npairs + 1) // 2)]
                for e in range(ECH):
                    xT = xpool.tile([P, L], BF16, tag="xT")
                    eng = nc.sync if e % 2 == 0 else nc.scalar
                    eng.dma_start(xT[:], x_dram[e * P:(e + 1) * P, :])
                    for d in range(2):
                        for it in range(L // IT):
                            pair = d * (L // IT) + it
                            tgt = accs[pair // 2][:, (pair % 2) * IT:(pair % 2 + 1) * IT]
                            nc.tensor.matmul(tgt, lhsT=w_sb[:, e, d * P:(d + 1) * P],
                                             rhs=xT[:, it * IT:(it + 1) * IT],
                                             start=(e == 0), stop=(e == ECH - 1))
                for d in range(2):
                    for it in range(L // IT):
                        pair = d * (L // IT) + it
                        src = accs[pair // 2][:, (pair % 2) * IT:(pair % 2 + 1) * IT]
                        dst_sl = dst[:, d, it * IT:(it + 1) * IT]
                        if it % 2 == 0:
                            nc.scalar.copy(dst_sl, src)
                        else:
                            nc.vector.tensor_copy(dst_sl, src)

            proj_qk(qb, wq_sb, qhT, QL)
            nc.scalar.dma_start(wk_sb[:], wkT.rearrange("(o p) d -> p o d", p=P))
            proj_qk(kb, wk_sb, khT, KL)
            nc.scalar.dma_start(wv_sb[:], wvT.rearrange("(o p) d -> p o d", p=P))

            # ---- v projection (out: [j part-major, head dim]) ----
            HALF = KL // 2
            for jg in range(2):
                njc_h = NJC // 2  # j-chunks in this half
                vaccs = [psum.tile([P, 2 * IT], F32, tag="ps", name=f"vacc{i}")
                         for i in range((njc_h + 1) // 2)]
                for e in range(ECH):
                    vT = xpool.tile([P, HALF], BF16, tag="xT")
                    eng = nc.sync if e % 2 == 0 else nc.scalar
                    eng.dma_start(
                        vT[:], vb[e * P:(e + 1) * P, jg * HALF:(jg + 1) * HALF])
                    for jc in range(njc_h):
                        tgt = vaccs[jc // 2][:, (jc % 2) * IT:(jc % 2) * IT + CDIM]
                        nc.tensor.matmul(tgt, lhsT=vT[:, jc * P:(jc + 1) * P],
                                         rhs=wv_sb[:, e, :],
                                         start=(e == 0), stop=(e == ECH - 1))
                for jc in range(njc_h):
                    j = jg * njc_h + jc
                    src = vaccs[jc // 2][:, (jc % 2) * IT:(jc % 2) * IT + CDIM]
                    nc.vector.tensor_copy(
                        vh[:, j, :, 0:HD],
                        src.rearrange("p (h c) -> p h c", h=NH))

            nc.scalar.dma_start(wo_sb[:], woT.rearrange("(o p) d -> p o d", p=P))

            # ---- phase 2: attention + fused softmax-rescale + out-proj ----
            # i-group outer (2 i-tiles = 1024 queries), heads inner; per
            # (h, ig) the inner j-loop is software-pipelined with LAG=2 so
            # the scalar-engine exp of chunk j overlaps the PV matmuls of
            # chunk j-2. The PV staging row 64 (softmax denominators) feeds
            # the custom-DVE reciprocal directly (partition 64 is a legal
            # start partition); the rescale happens before the partition-
            # redistributing DMA into the out-proj operand layout.
            LAG = 0
            attnTb = [wpool.tile([P, CDIM // P, 2 * IT], BF16, tag=f"attnTb{ig}",
                                 name=f"attnTb{ig}")
                      for ig in range(NIT // 2)]

            def emit_outproj(ig, icws):
                for icw in icws:
                    ic = ig * (2 * IT // P) + icw
                    po = psum.tile([P, 2 * IT], F32, tag="ps", name="po")
                    for ec in range(CDIM // P):
                        for dt in range(2):
                            nc.tensor.matmul(
                                po[:, dt * IT:(dt + 1) * IT],
                                lhsT=attnTb[ig][:, ec, icw * P:(icw + 1) * P],
                                rhs=wo_sb[:, ec, dt * IT:(dt + 1) * IT],
                                start=(ec == 0), stop=(ec == CDIM // P - 1))
                    for dt in range(2):
                        ob = opool.tile([P, IT], F32, tag="ob")
                        if (icw + dt) % 2 == 0:
                            nc.scalar.copy(ob[:], po[:, dt * IT:(dt + 1) * IT])
                        else:
                            nc.vector.tensor_copy(ob[:], po[:, dt * IT:(dt + 1) * IT])
                        nc.scalar.dma_start(
                            outp[ic * P:(ic + 1) * P, dt * IT:(dt + 1) * IT], ob[:])
            for ig in range(NIT // 2):
                for h in range(NH):
                    hp, hh = h // 2, h % 2
                    q_h = qhT[hh * HD:(hh + 1) * HD, hp, :]
                    k_h = khT[hh * HD:(hh + 1) * HD, hp, :]
                    pv = psum.tile([P, 2 * IT], F32, tag="ps", name="pv")
                    Pts = {}
                    NB = NJC // 2  # batches of 2 j-chunks, PV trails one batch
                    for b in range(NB + 1):
                        if b < NB:
                            pss = []
                            for jo in range(2):
                                j = 2 * b + jo
                                ps = psum.tile([P, 2 * IT], F32, tag="ps",
                                               name="s")
                                pss.append(ps)
                                for itp in range(2):
                                    it = ig * 2 + itp
                                    nc.tensor.matmul(
                                        ps[:, itp * IT:(itp + 1) * IT],
                                        lhsT=k_h[:, j * P:(j + 1) * P],
                                        rhs=q_h[:, it * IT:(it + 1) * IT],
                                        start=True, stop=True)
                            for jo in range(2):
                                j = 2 * b + jo
                                Pt = ppool.tile([P, 2, IT], BF16, tag="Pt")
                                nc.scalar.activation(
                                    Pt[:],
                                    pss[jo][:].rearrange("p (a b) -> p a b", a=2),
                                    mybir.ActivationFunctionType.Exp, scale=SCALE)
                                Pts[j] = Pt
                        if b >= 1:
                            for jo in range(2):
                                jj = 2 * (b - 1) + jo
                                Pt = Pts.pop(jj)
                                for itp in range(2):
                                    nc.tensor.matmul(
                                        pv[0:HD + 1, itp * IT:itp * IT + IT],
                                        lhsT=vh[:, jj, h, :], rhs=Pt[:, itp, :],
                                        start=(jj == 0), stop=(jj == NJC - 1))
                    # evacuate + rescale: st rows 0..63 = PV, row 64 = sums
                    st = stpool.tile([P, 2 * IT], F32, tag="pvstage")
                    nc.vector.tensor_copy(st[0:HD + 1, :], pv[0:HD + 1, :])
                    s0 = rpool.tile([1, 2 * IT], F32, tag="s0")
                    nc.sync.dma_start(s0[:], st[HD:HD + 1, :])
                    rrec = rpool.tile([1, 2 * IT], F32, tag="rrec")
                    rscr = rpool.tile([1, 2 * IT], F32, tag="rscr")
                    nc.vector.reciprocal_approx_accurate(
                        out=rrec[:], in_=s0[:], scratch=rscr[:])
                    dtmp = dpool.tile([1, 2 * IT], F32, tag="dtmp")
                    nc.sync.dma_start(dtmp[:], rrec[:])
                    rbc = rpool.tile([HD, 2 * IT], F32, tag="rbc")
                    nc.sync.dma_start(rbc[:], dtmp[0:1, :].broadcast_to((HD, 2 * IT)))
                    stb = stpool.tile([HD, 2 * IT], BF16, tag="stb")
                    nc.vector.tensor_mul(stb[:], st[0:HD, :], rbc[:])
                    nc.sync.dma_start(
                        attnTb[ig][hh * HD:(hh + 1) * HD, hp, :], stb[:])

                    if ig > 0:
                        emit_outproj(ig - 1, range(2 * h, 2 * h + 2))
                if ig == NIT // 2 - 1:
                    emit_outproj(ig, range(2 * IT // P))

    nc.compile()
    return nc


_NC_CACHE = {}


def _get_nc(QL, KL):
    key = (QL, KL)
    if key not in _NC_CACHE:
        _NC_CACHE[key] = build_bass(QL, KL)
    return _NC_CACHE[key]


def make_in_maps(q, k, v, Wq, Wk, Wv, Wo):
    """Per-core input maps (bf16, weights pre-transposed)."""
    bf = ml_dtypes.bfloat16
    q, k, v = (np.asarray(x, np.float32) for x in (q, k, v))
    WqT = np.asarray(Wq, np.float32).T.astype(bf)
    WkT = np.asarray(Wk, np.float32).T.astype(bf)
    WvT = np.asarray(Wv, np.float32).T.astype(bf)
    WoT = np.asarray(Wo, np.float32).T.astype(bf)
    qb = [np.ascontiguousarray(q[b].T.astype(bf)) for b in range(B)]
    kb = [np.ascontiguousarray(k[b].T.astype(bf)) for b in range(B)]
    vb = [np.ascontiguousarray(v[b].T.astype(bf)) for b in range(B)]
    in_maps = []
    for c in range(NCORES):
        b, hs = c // 4, c % 4
        sl = slice(hs * CDIM, (hs + 1) * CDIM)
        in_maps.append({
            "qbT": qb[b], "kbT": kb[b], "vbT": vb[b],
            "wqT": np.ascontiguousarray(WqT[:, sl]),
            "wkT": np.ascontiguousarray(WkT[:, sl]),
            "wvT": np.ascontiguousarray(WvT[:, sl]),
            "woT": np.ascontiguousarray(WoT[sl, :]),
        })
    return in_maps


def kernel(q, k, v, Wq, Wk, Wv, Wo, bo, _trace=False):
    q = np.asarray(q, np.float32)
    QL, KL = q.shape[1], np.asarray(k).shape[1]
    nc = _get_nc(QL, KL)
    in_maps = make_in_maps(q, k, v, Wq, Wk, Wv, Wo)
    res = run_bass_kernel_spmd(nc, in_maps, core_ids=list(range(NCORES)),
                               trace=_trace)
    bo = np.asarray(bo, np.float32)
    out = np.empty((B, QL, DIM), np.float32)
    for b in range(B):
        acc = res.results[4 * b]["outp"].copy()
        for c in range(4 * b + 1, 4 * b + 4):
            acc += res.results[c]["outp"]
        out[b] = acc + bo
    if _trace:
        kernel._last_results = res
    return out


# revision 31
# speedup vs baseline: 1.4426x; 1.0178x over previous
"""Multi-head attention (B=2, QL=KL=2048, DIM=1024, H=16) on 8 TRN2 NeuronCores.

Sharding: core c handles batch c//4 and heads (c%4)*4 .. (c%4)*4+4 (column-
parallel q/k/v projections, row-parallel out projection). Each core emits a
partial output [QL, DIM]; the host sums the 4 partials per batch and adds the
output bias (the row-parallel all-reduce, done at unshard time).

Per-core kernel layout (all matmul operands bf16, fp32 PSUM accumulation):
  - activations are loaded feature-major (x^T tiles) via XBAR DMA-transpose
    (host pre-casts q/k/v to bf16; fp32 has no DMA-transpose path)
  - weights arrive host-pre-transposed (WqT etc), so no on-chip transposes
  - scores are computed transposed per head: S^T[j,i] = khT.T @ qhT (K=64)
  - exp(SCALE * S) is fused into the PSUM evacuation on the scalar engine
  - PV uses a ones-augmented V (lhsT [j, 65]) so row 64 of the PSUM output
    accumulates the softmax denominators for free
  - 1/sums via the 2-ULP custom-DVE reciprocal; the scale is applied to the
    fp32 attention output while casting it to bf16 for the out-projection
"""

import numpy as np
import ml_dtypes

import concourse.bass as bass
import concourse.mybir as mybir
import concourse.tile as tile
from concourse import bacc
from concourse.bass_utils import run_bass_kernel_spmd

BF16 = mybir.dt.bfloat16
F32 = mybir.dt.float32

B = 2
DIM = 1024
NUM_HEADS = 16
HD = DIM // NUM_HEADS  # 64
SCALE = HD ** -0.5
NCORES = 8
NH = 4          # heads per core
CDIM = NH * HD  # 256, per-core slice of the head dim
P = 128
IT = 512        # i (query) tile
ECH = DIM // P  # 8 contraction chunks for the projections


def build_bass(QL=2048, KL=2048, num_devices=NCORES):
    assert QL % IT == 0 and KL % 256 == 0
    NIT = QL // IT
    NJC = KL // P  # j (key) chunks

    nc = bacc.Bacc("TRN2", target_bir_lowering=False, debug=False,
                   num_devices=num_devices)
    qb = nc.dram_tensor("qbT", [DIM, QL], BF16, kind="ExternalInput").ap()
    kb = nc.dram_tensor("kbT", [DIM, KL], BF16, kind="ExternalInput").ap()
    vb = nc.dram_tensor("vbT", [DIM, KL], BF16, kind="ExternalInput").ap()
    wqT = nc.dram_tensor("wqT", [DIM, CDIM], BF16, kind="ExternalInput").ap()
    wkT = nc.dram_tensor("wkT", [DIM, CDIM], BF16, kind="ExternalInput").ap()
    wvT = nc.dram_tensor("wvT", [DIM, CDIM], BF16, kind="ExternalInput").ap()
    woT = nc.dram_tensor("woT", [CDIM, DIM], BF16, kind="ExternalInput").ap()
    outp = nc.dram_tensor("outp", [QL, DIM], F32, kind="ExternalOutput").ap()

    with tile.TileContext(nc) as tc:
        with (
            tc.tile_pool(name="wpool", bufs=1) as wpool,
            tc.tile_pool(name="xpool", bufs=8) as xpool,
            tc.tile_pool(name="ppool", bufs=4) as ppool,
            tc.tile_pool(name="stpool", bufs=3) as stpool,
            tc.tile_pool(name="rpool", bufs=3) as rpool,
            tc.tile_pool(name="opool", bufs=4) as opool,
            tc.tile_pool(name="dpool", bufs=8, space="DRAM") as dpool,
            tc.tile_pool(name="psum", bufs=4, space="PSUM") as psum,
        ):
            # ---- persistent SBUF tensors ----
            wq_sb = wpool.tile([P, ECH, CDIM], BF16, tag="wq")
            wk_sb = wpool.tile([P, ECH, CDIM], BF16, tag="wk")
            wv_sb = wpool.tile([P, ECH, CDIM], BF16, tag="wv")
            wo_sb = wpool.tile([P, CDIM // P, DIM], BF16, tag="wo")
            nc.scalar.dma_start(wq_sb[:], wqT.rearrange("(o p) d -> p o d", p=P))

            qhT = wpool.tile([P, CDIM // P, QL], BF16, tag="qhT")
            khT = wpool.tile([P, CDIM // P, KL], BF16, tag="khT")
            vh = wpool.tile([P, NJC, NH, HD + 1], BF16, tag="vh")
            nc.gpsimd.memset(vh[:, :, :, HD], 1.0)  # ones column -> row sums

            # ---- phase 1: q/k projections (out: [d'(256) part-major, token]) ----
            def proj_qk(x_dram, w_sb, dst, L):
                npairs = 2 * (L // IT)
                accs = [psum.tile([P, 2 * IT], F32, tag="ps", name=f"acc{i}")
                         for i in range((npairs + 1) // 2)]
                for e in range(ECH):
                    xT = xpool.tile([P, L], BF16, tag="xT")
                    eng = nc.sync if e % 2 == 0 else nc.scalar
                    eng.dma_start(xT[:], x_dram[e * P:(e + 1) * P, :])
                    for d in range(2):
                        for it in range(L // IT):
                            pair = d * (L // IT) + it
                            tgt = accs[pair // 2][:, (pair % 2) * IT:(pair % 2 + 1) * IT]
                            nc.tensor.matmul(tgt, lhsT=w_sb[:, e, d * P:(d + 1) * P],
                                             rhs=xT[:, it * IT:(it + 1) * IT],
                                             start=(e == 0), stop=(e == ECH - 1))
                for d in range(2):
                    for it in range(L // IT):
                        pair = d * (L // IT) + it
                        src = accs[pair // 2][:, (pair % 2) * IT:(pair % 2 + 1) * IT]
                        dst_sl = dst[:, d, it * IT:(it + 1) * IT]
                        if it % 2 == 0:
                            nc.scalar.copy(dst_sl, src)
                        else:
                            nc.vector.tensor_copy(dst_sl, src)

            proj_qk(qb, wq_sb, qhT, QL)
            nc.scalar.dma_start(wk_sb[:], wkT.rearrange("(o p) d -> p o d", p=P))
            proj_qk(kb, wk_sb, khT, KL)
            nc.scalar.dma_start(wv_sb[:], wvT.rearrange("(o p) d -> p o d", p=P))

            # ---- v projection (out: [j part-major, head dim]) ----
            HALF = KL // 2
            for jg in range(2):
                njc_h = NJC // 2  # j-chunks in this half
                vaccs = [psum.tile([P, 2 * IT], F32, tag="ps", name=f"vacc{i}")
                         for i in range((njc_h + 1) // 2)]
                for e in range(ECH):
                    vT = xpool.tile([P, HALF], BF16, tag="xT")
                    eng = nc.sync if e % 2 == 0 else nc.scalar
                    eng.dma_start(
                        vT[:], vb[e * P:(e + 1) * P, jg * HALF:(jg + 1) * HALF])
                    for jc in range(njc_h):
                        tgt = vaccs[jc // 2][:, (jc % 2) * IT:(jc % 2) * IT + CDIM]
                        nc.tensor.matmul(tgt, lhsT=vT[:, jc * P:(jc + 1) * P],
                                         rhs=wv_sb[:, e, :],
                                         start=(e == 0), stop=(e == ECH - 1))
                for jc in range(njc_h):
                    j = jg * njc_h + jc
                    src = vaccs[jc // 2][:, (jc % 2) * IT:(jc % 2) * IT + CDIM]
                    nc.vector.tensor_copy(
                        vh[:, j, :, 0:HD],
                        src.rearrange("p (h c) -> p h c", h=NH))

            nc.scalar.dma_start(wo_sb[:], woT.rearrange("(o p) d -> p o d", p=P))

            # ---- phase 2: attention + fused softmax-rescale + out-proj ----
            # i-group outer (2 i-tiles = 1024 queries), heads inner; per
            # (h, ig) the inner j-loop is software-pipelined with LAG=2 so
            # the scalar-engine exp of chunk j overlaps the PV matmuls of
            # chunk j-2. The PV staging row 64 (softmax denominators) feeds
            # the custom-DVE reciprocal directly (partition 64 is a legal
            # start partition); the rescale happens before the partition-
            # redistributing DMA into the out-proj operand layout.
            LAG = 0
            attnTb = [wpool.tile([P, CDIM // P, 2 * IT], BF16, tag=f"attnTb{ig}",
                                 name=f"attnTb{ig}")
                      for ig in range(NIT // 2)]

            def emit_outproj(ig, icws):
                for icw in icws:
                    ic = ig * (2 * IT // P) + icw
                    po = psum.tile([P, 2 * IT], F32, tag="ps", name="po")
                    for ec in range(CDIM // P):
                        for dt in range(2):
                            nc.tensor.matmul(
                                po[:, dt * IT:(dt + 1) * IT],
                                lhsT=attnTb[ig][:, ec, icw * P:(icw + 1) * P],
                                rhs=wo_sb[:, ec, dt * IT:(dt + 1) * IT],
                                start=(ec == 0), stop=(ec == CDIM // P - 1))
                    for dt in range(2):
                        ob = opool.tile([P, IT], F32, tag="ob")
                        if (icw + dt) % 2 == 0:
                            nc.scalar.copy(ob[:], po[:, dt * IT:(dt + 1) * IT])
                        else:
                            nc.vector.tensor_copy(ob[:], po[:, dt * IT:(dt + 1) * IT])
                        nc.scalar.dma_start(
                            outp[ic * P:(ic + 1) * P, dt * IT:(dt + 1) * IT], ob[:])
            pending_tail = []

            def emit_block(ig, h):
                hp, hh = h // 2, h % 2
                q_h = qhT[hh * HD:(hh + 1) * HD, hp, :]
                k_h = khT[hh * HD:(hh + 1) * HD, hp, :]
                pv = psum.tile([P, 2 * IT], F32, tag="ps", name="pv")
                Pts = {}
                NB = NJC // 2  # batches of 2 j-chunks, PV trails one batch
                for b in range(NB):
                    pss = []
                    for jo in range(2):
                        j = 2 * b + jo
                        ps = psum.tile([P, 2 * IT], F32, tag="ps", name="s")
                        pss.append(ps)
                        for itp in range(2):
                            it = ig * 2 + itp
                            nc.tensor.matmul(
                                ps[:, itp * IT:(itp + 1) * IT],
                                lhsT=k_h[:, j * P:(j + 1) * P],
                                rhs=q_h[:, it * IT:(it + 1) * IT],
                                start=True, stop=True)
                    for jo in range(2):
                        j = 2 * b + jo
                        Pt = ppool.tile([P, 2, IT], BF16, tag="Pt")
                        nc.scalar.activation(
                            Pt[:], pss[jo][:].rearrange("p (a b) -> p a b", a=2),
                            mybir.ActivationFunctionType.Exp, scale=SCALE)
                        Pts[j] = Pt
                    if b == 0 and pending_tail:
                        pending_tail.pop(0)()  # previous block's tail
                    if b >= 1:
                        for jo in range(2):
                            jj = 2 * (b - 1) + jo
                            Pt = Pts.pop(jj)
                            for itp in range(2):
                                nc.tensor.matmul(
                                    pv[0:HD + 1, itp * IT:itp * IT + IT],
                                    lhsT=vh[:, jj, h, :], rhs=Pt[:, itp, :],
                                    start=(jj == 0), stop=False)

                def tail(ig=ig, h=h, hp=hp, hh=hh, pv=pv, Pts=Pts):
                    for jo in range(2):
                        jj = NJC - 2 + jo
                        Pt = Pts.pop(jj)
                        for itp in range(2):
                            nc.tensor.matmul(
                                pv[0:HD + 1, itp * IT:itp * IT + IT],
                                lhsT=vh[:, jj, h, :], rhs=Pt[:, itp, :],
                                start=False, stop=(jj == NJC - 1))
                    # evacuate + rescale: st rows 0..63 = PV, row 64 = sums
                    st = stpool.tile([P, 2 * IT], F32, tag="pvstage")
                    nc.vector.tensor_copy(st[0:HD + 1, :], pv[0:HD + 1, :])
                    s0 = rpool.tile([1, 2 * IT], F32, tag="s0")
                    nc.gpsimd.dma_start(s0[:], st[HD:HD + 1, :])
                    rrec = rpool.tile([1, 2 * IT], F32, tag="rrec")
                    rscr = rpool.tile([1, 2 * IT], F32, tag="rscr")
                    nc.vector.reciprocal_approx_accurate(
                        out=rrec[:], in_=s0[:], scratch=rscr[:])
                    dtmp = dpool.tile([1, 2 * IT], F32, tag="dtmp")
                    nc.gpsimd.dma_start(dtmp[:], rrec[:])
                    rbc = rpool.tile([HD, 2 * IT], F32, tag="rbc")
                    nc.gpsimd.dma_start(rbc[:],
                                        dtmp[0:1, :].broadcast_to((HD, 2 * IT)))
                    stb = stpool.tile([HD, 2 * IT], BF16, tag="stb")
                    nc.vector.tensor_mul(stb[:], st[0:HD, :], rbc[:])
                    nc.sync.dma_start(
                        attnTb[ig][hh * HD:(hh + 1) * HD, hp, :], stb[:])

                pending_tail.append(tail)

            for ig in range(NIT // 2):
                for h in range(NH):
                    emit_block(ig, h)
                    if ig > 0:
                        emit_outproj(ig - 1, range(2 * h, 2 * h + 2))
                if ig == NIT // 2 - 1:
                    while pending_tail:
                        pending_tail.pop(0)()
                    emit_outproj(ig, range(2 * IT // P))

    nc.compile()
    return nc


_NC_CACHE = {}


def _get_nc(QL, KL):
    key = (QL, KL)
    if key not in _NC_CACHE:
        _NC_CACHE[key] = build_bass(QL, KL)
    return _NC_CACHE[key]


def make_in_maps(q, k, v, Wq, Wk, Wv, Wo):
    """Per-core input maps (bf16, weights pre-transposed)."""
    bf = ml_dtypes.bfloat16
    q, k, v = (np.asarray(x, np.float32) for x in (q, k, v))
    WqT = np.asarray(Wq, np.float32).T.astype(bf)
    WkT = np.asarray(Wk, np.float32).T.astype(bf)
    WvT = np.asarray(Wv, np.float32).T.astype(bf)
    WoT = np.asarray(Wo, np.float32).T.astype(bf)
    qb = [np.ascontiguousarray(q[b].T.astype(bf)) for b in range(B)]
    kb = [np.ascontiguousarray(k[b].T.astype(bf)) for b in range(B)]
    vb = [np.ascontiguousarray(v[b].T.astype(bf)) for b in range(B)]
    in_maps = []
    for c in range(NCORES):
        b, hs = c // 4, c % 4
        sl = slice(hs * CDIM, (hs + 1) * CDIM)
        in_maps.append({
            "qbT": qb[b], "kbT": kb[b], "vbT": vb[b],
            "wqT": np.ascontiguousarray(WqT[:, sl]),
            "wkT": np.ascontiguousarray(WkT[:, sl]),
            "wvT": np.ascontiguousarray(WvT[:, sl]),
            "woT": np.ascontiguousarray(WoT[sl, :]),
        })
    return in_maps


def kernel(q, k, v, Wq, Wk, Wv, Wo, bo, _trace=False):
    q = np.asarray(q, np.float32)
    QL, KL = q.shape[1], np.asarray(k).shape[1]
    nc = _get_nc(QL, KL)
    in_maps = make_in_maps(q, k, v, Wq, Wk, Wv, Wo)
    res = run_bass_kernel_spmd(nc, in_maps, core_ids=list(range(NCORES)),
                               trace=_trace)
    bo = np.asarray(bo, np.float32)
    out = np.empty((B, QL, DIM), np.float32)
    for b in range(B):
        acc = res.results[4 * b]["outp"].copy()
        for c in range(4 * b + 1, 4 * b + 4):
            acc += res.results[c]["outp"]
        out[b] = acc + bo
    if _trace:
        kernel._last_results = res
    return out
